# revision 3
# baseline (speedup 1.0000x reference)
"""MoE SwiGLU experts (T=2048, H=2048, I=5632, E=8, top-2) on 8 trn2 cores.

Expert-parallel routed compute, one expert per NeuronCore, f32r matmuls
(HW-measured 227ns/MM at N=512 vs bf16's 255ns/MM — f32r hides its
LDWEIGHTS better).  vs the original f32r kernel this keeps the matmul
stream gapless at the edges:
 - exact capacity c (max expert load, ceil to 8) instead of ceil-128
 - input DMAs split across two HWDGE queues (Sync: w1/w3 stream;
   Scalar: xg, w2, outputs) so first tiles land while the PE warms up,
   and no input DMA queues behind the warmup scratch store (emitted last)
 - phase 1 uses 4 PSUM banks per icg group (22 groups of wt_w=256), so
   consecutive groups double-buffer across the 8 banks and the silu/mul
   eviction overlaps the next group's matmuls
 - phase 2 is output-transposed: yT[h] accumulates in ONE psum bank per
   128-row H-tile (44-chunk chain), evicted while the next tile streams;
   only the last tile's copy+store is in the kernel tail

Per core (expert e), capacity c:
  phase 1: hT[i, t] = silu(w1[e].T @ xgT) * (w3[e].T @ xgT)   [I, c]
  phase 2: yT[h, t] = sum_ik w2[e][ik, h].T @ hT[ik]          [H, c]
Host: gather tokens per expert (merge duplicate top-k), retile weights
so every DMA is contiguous; scatter-add outputs with router weights.
"""

import numpy as np

import concourse.bacc as bacc
import concourse.mybir as mybir
import concourse.tile as tile
from concourse.bass_utils import run_bass_kernel_spmd

E = 8
H = 2048
I = 5632
HK = H // 128     # 16 contraction chunks for phase 1
IK = I // 128     # 44 contraction chunks for phase 2
HT = H // 128     # 16 output row tiles for phase 2
WT = 256          # phase-1 weight group width (I cols)
ICPT = WT // 128  # 2 ic-tiles per phase-1 weight group
NICG = I // WT    # 22 phase-1 weight groups
HP = 4            # hk values packed per weight DMA tile
IKH = IK // 2     # 22: w2 loaded in two half tiles per h

F32 = mybir.dt.float32
F32R = mybir.dt.float32r
SILU = mybir.ActivationFunctionType.Silu

_prog_cache: dict[int, object] = {}


def _chunks(c):
    """Moving-dim chunks of at most 512 (PSUM bank limit), all >=256 when
    c > 512 so f32r stays at full PE rate."""
    if c <= 512:
        return [(0, c)]
    c1 = -(-c // 16) * 8
    return [(0, c1), (c1, c - c1)]


def _build(c):
    nc = bacc.Bacc("TRN2", target_bir_lowering=False, debug=False, num_devices=E)
    xgT = nc.dram_tensor("xgT", [HP, 128, 4, c], F32R, kind="ExternalInput")
    w1 = nc.dram_tensor("w1", [NICG, HP, 128, 4, WT], F32R, kind="ExternalInput")
    w3 = nc.dram_tensor("w3", [NICG, HP, 128, 4, WT], F32R, kind="ExternalInput")
    w2 = nc.dram_tensor("w2", [HT, 128, IK, 128], F32R, kind="ExternalInput")
    yT = nc.dram_tensor("yT", [HT, 128, c], F32, kind="ExternalOutput")
    scratch = nc.dram_tensor("scratch", [128, 256], F32R, kind="ExternalOutput")

    ch = _chunks(c)

    with tile.TileContext(nc) as tc:
        with (
            tc.tile_pool(name="xg", bufs=HP) as xpool,
            tc.tile_pool(name="h", bufs=IK) as hpool,
            tc.tile_pool(name="w", bufs=3) as wpool,
            tc.tile_pool(name="w2p", bufs=3) as w2pool,
            tc.tile_pool(name="ps", bufs=8, space="PSUM") as pspool,
            tc.tile_pool(name="o", bufs=3) as opool,
            tc.tile_pool(name="wu", bufs=1) as wupool,
        ):
            # PE warmup on a zeroed tile while the first input DMAs land.
            # The scratch store keeping it alive is emitted at the END so
            # nothing queues behind it.
            wu0 = wupool.tile([128, 256], F32, tag="wu0", name="wu0")
            nc.vector.memset(wu0[:], 0.0)
            wu = wupool.tile([128, 256], F32R, tag="wu", name="wu")
            nc.vector.tensor_copy(wu[:], wu0[:])
            wups = pspool.tile([128, 256], F32, tag="ps", name="wups")
            for _ in range(12):
                nc.tensor.matmul(wups[:], wu[:, :128], wu[:], start=True, stop=True)
            wuo = opool.tile([128, 256], F32R, tag="o", name="wuo")
            nc.vector.tensor_copy(wuo[:], wups[:])

            # Gathered tokens: 4 batched tiles of 4 hk chunks each, issued
            # on the Scalar HWDGE queue so they overlap the Sync-queue
            # weight stream.
            xg = []
            for hp in range(HP):
                t = xpool.tile([128, 4, c], F32R, tag="xg", name=f"xg{hp}")
                nc.scalar.dma_start(t[:], xgT[hp])
                xg.append(t)
            hT = [
                hpool.tile([128, c], F32R, tag="h", name=f"h{ik}")
                for ik in range(IK)
            ]

            # phase 1: hT = silu(w1.T @ xgT) * (w3.T @ xgT)
            for icg in range(NICG):
                ps = {}
                for w in (0, 1):
                    for ic in range(ICPT):
                        for ci, (off, sz) in enumerate(ch):
                            ps[w, ic, ci] = pspool.tile(
                                [128, sz], F32, tag="ps", name=f"ps{w}_{ic}_{ci}"
                            )
                for hp in range(HP):
                    wt1 = wpool.tile(
                        [128, 4, WT], F32R, tag="w1", name=f"w1_{icg}_{hp}"
                    )
                    nc.sync.dma_start(wt1[:], w1[icg, hp])
                    wt3 = wpool.tile(
                        [128, 4, WT], F32R, tag="w3", name=f"w3_{icg}_{hp}"
                    )
                    nc.sync.dma_start(wt3[:], w3[icg, hp])
                    for hh in range(4):
                        hk = hp * 4 + hh
                        for w, wt in ((0, wt1), (1, wt3)):
                            for ic in range(ICPT):
                                for ci, (off, sz) in enumerate(ch):
                                    nc.tensor.matmul(
                                        ps[w, ic, ci][:],
                                        wt[:, hh, ic * 128 : (ic + 1) * 128],
                                        xg[hp][:, hh, off : off + sz],
                                        start=(hk == 0),
                                        stop=(hk == HK - 1),
                                    )
                for ic in range(ICPT):
                    ik = icg * ICPT + ic
                    for ci, (off, sz) in enumerate(ch):
                        dst = hT[ik][:, off : off + sz]
                        nc.scalar.activation(dst, ps[0, ic, ci][:], SILU)
                        nc.vector.tensor_mul(dst, dst, ps[1, ic, ci][:])

            # phase 2: yT[h] = sum_ik w2[h, ik].T @ hT[ik]
            for h in range(HT):
                w2a = w2pool.tile([128, IKH, 128], F32R, tag="w2", name=f"w2a_{h}")
                nc.scalar.dma_start(w2a[:], w2[h, :, 0:IKH])
                w2b = w2pool.tile([128, IKH, 128], F32R, tag="w2", name=f"w2b_{h}")
                nc.scalar.dma_start(w2b[:], w2[h, :, IKH:IK])
                for ci, (off, sz) in enumerate(ch):
                    ps2 = pspool.tile([128, sz], F32, tag="ps", name=f"ps2_{h}_{ci}")
                    for ik in range(IK):
                        wt = w2a if ik < IKH else w2b
                        nc.tensor.matmul(
                            ps2[:],
                            wt[:, ik % IKH, :],
                            hT[ik][:, off : off + sz],
                            start=(ik == 0),
                            stop=(ik == IK - 1),
                        )
                    ot = opool.tile([128, sz], F32, tag="o", name=f"o{h}_{ci}")
                    nc.vector.tensor_copy(ot[:], ps2[:])
                    nc.scalar.dma_start(yT[h, :, off : off + sz], ot[:])

            nc.scalar.dma_start(scratch[:], wuo[:])
    nc.compile()
    return nc


def _get_prog(c):
    if c not in _prog_cache:
        _prog_cache[c] = _build(c)
    return _prog_cache[c]


def _retile_weights(w1, w2, w3):
    """Retile so every device DMA is one contiguous block."""
    w1 = np.asarray(w1, np.float32)
    w3 = np.asarray(w3, np.float32)
    w2 = np.asarray(w2, np.float32)
    # [E, NICG, HP, 128, 4, WT]: (e,icg,hp,p,hh,i) = w[e,(hp*4+hh)*128+p, icg*WT+i]
    w1t = np.ascontiguousarray(
        w1.reshape(E, HP, 4, 128, NICG, WT).transpose(0, 4, 1, 3, 2, 5)
    )
    w3t = np.ascontiguousarray(
        w3.reshape(E, HP, 4, 128, NICG, WT).transpose(0, 4, 1, 3, 2, 5)
    )
    # [E, HT, 128, IK, 128]: (e,h,p,ik,j) = w2[e, ik*128+p, h*128+j]
    w2t = np.ascontiguousarray(
        w2.reshape(E, IK, 128, HT, 128).transpose(0, 3, 2, 1, 4)
    )
    return w1t, w3t, w2t


def kernel(x, expert_weights, w1, w2, w3, expert_indices):
    x = np.asarray(x, dtype=np.float32)
    expert_weights = np.asarray(expert_weights, dtype=np.float32)
    idx = np.asarray(expert_indices)
    T = x.shape[0]

    # Route: token lists per expert, merging duplicate top-k hits so each
    # token appears at most once per expert (scatter-add safe).
    same = idx[:, 0] == idx[:, 1]
    w_slot0 = np.where(same, expert_weights[:, 0] + expert_weights[:, 1],
                       expert_weights[:, 0])
    toks, wts = [], []
    for e in range(E):
        m0 = idx[:, 0] == e
        m1 = (idx[:, 1] == e) & ~same
        t0 = np.nonzero(m0)[0]
        t1 = np.nonzero(m1)[0]
        toks.append(np.concatenate([t0, t1]))
        wts.append(np.concatenate([w_slot0[m0], expert_weights[m1, 1]]))

    maxcount = max(max(len(t) for t in toks), 1)
    nrounds = -(-maxcount // 1024)
    c = -(-(-(-maxcount // nrounds)) // 8) * 8  # ceil to 8
    c = max(c, 128)

    w1t, w3t, w2t = _retile_weights(w1, w2, w3)
    nc = _get_prog(c)

    out = np.zeros((T, H), dtype=np.float32)
    for r in range(nrounds):
        in_maps = []
        seg_toks = []
        seg_wts = []
        for e in range(E):
            seg = toks[e][r * c : (r + 1) * c]
            sw = wts[e][r * c : (r + 1) * c]
            seg_toks.append(seg)
            seg_wts.append(sw)
            xga = np.zeros((H, c), dtype=np.float32)
            if len(seg):
                xga[:, : len(seg)] = x[seg].T
            # [HP, 128, 4, c]: (hp, p, hh, t) = xga[(hp*4+hh)*128+p, t]
            in_maps.append(
                {
                    "xgT": np.ascontiguousarray(
                        xga.reshape(HP, 4, 128, c).transpose(0, 2, 1, 3)
                    ),
                    "w1": w1t[e],
                    "w3": w3t[e],
                    "w2": w2t[e],
                }
            )
        res = run_bass_kernel_spmd(nc, in_maps, core_ids=list(range(E)))
        for e in range(E):
            seg = seg_toks[e]
            if len(seg) == 0:
                continue
            ye = res.results[e]["yT"].reshape(H, c).T[: len(seg)]
            out[seg] += ye * seg_wts[e][:, None]
    return out
